# revision 5
# baseline (speedup 1.0000x reference)
"""ConvTranspose2D (stride 2, pad 1, k 4) on 8 Trainium2 NeuronCores.

x: (256, 128, 128) f32, W: (128, 256, 4, 4), b: (128,) -> out: (128, 256, 256)

Decomposition: each output-pixel parity class (row even/odd x col even/odd)
is a dense 2x2-tap stride-1 conv over x. Output rows are sharded across the
8 cores (32 rows each); every core runs the same Bass program on its own
input slice, contraction over C_in=256 runs as 2 chunks of 128 partitions.

  even out rows: out[2m]   = K[0] x[m-1] + K[2] x[m]
  odd  out rows: out[2m+1] = K[1] x[m]   + K[3] x[m+1]
(and identically over columns), so tap (dy,dx) of parity (ry,rx) uses
kernel element ky=2*dy+ry, kx=2*dx+rx at input row m+dy+ry-1,
col n+dx+rx-1 -- handled by a zero-padded per-core input slab.
"""

import json

import numpy as np

import concourse.bass as bass
import concourse.bass2jax as bass2jax
import concourse.bass_utils as bass_utils
import concourse.tile as tile
from concourse import mybir
from concourse.bass_utils import run_bass_kernel_spmd

N_CORES = 8
C_IN = 256
C_OUT = 128
H = 128
W_IN = 128
ROWS_PER_CORE = H // N_CORES          # 16 input rows -> 32 output rows
MB = 4                                # m-blocks per core (4 rows each)
PAR = [(0, 0), (0, 1), (1, 0), (1, 1)]  # (row parity, col parity)
XW = W_IN + 2                         # zero-padded input width

_MATMUL_DT = mybir.dt.float32r


def _split_excess_waits(bir_json: bytes) -> bytes:
    # walrus CoreV3 codegen on this build supports only ONE embedded
    # sync-wait per instruction; Tile emits several. Hoist the excess onto
    # standalone EventSemaphore instructions on the same engine just before
    # the original instruction (same semantics: the sequencer blocks on
    # each in turn).
    bir = json.loads(bir_json)
    changed = False
    for fn in bir.get("functions", []):
        for bb in fn.get("blocks", []):
            out = []
            for inst in bb.get("instructions", []):
                si = inst.get("sync_info")
                if si:
                    ow = si.get("on_wait") or []
                    if len(ow) > 1:
                        for j, w in enumerate(ow[:-1]):
                            out.append({
                                "debug": inst.get("debug", 0),
                                "engine": inst["engine"],
                                "ins": [],
                                "name": f"{inst['name']}-wsplit{j}",
                                "opcode": "EventSemaphore",
                                "outs": [],
                                "sync_info": {"on_update": [], "on_wait": [w]},
                            })
                        si["on_wait"] = ow[-1:]
                        changed = True
                out.append(inst)
            bb["instructions"] = out
    if not changed:
        return bir_json
    return json.dumps(bir).encode()


_orig_compile_bir_kernel = bass_utils.compile_bir_kernel


def _patched_compile_bir_kernel(bir_json, tmpdir, neff_name="file.neff"):
    return _orig_compile_bir_kernel(_split_excess_waits(bir_json), tmpdir, neff_name)


bass_utils.compile_bir_kernel = _patched_compile_bir_kernel
bass2jax.compile_bir_kernel = _patched_compile_bir_kernel


def _build_bass():
    nc = bass.Bass(trn_type="TRN2")
    xh = nc.declare_dram_parameter("xh", [128, 2, 18, XW], _MATMUL_DT, isOutput=False)
    wt = nc.declare_dram_parameter("wt", [128, 4, 2, 2, 2, C_OUT], _MATMUL_DT, isOutput=False)
    bh = nc.declare_dram_parameter("bh", [C_OUT, 1], mybir.dt.float32, isOutput=False)
    out = nc.declare_dram_parameter("out", [C_OUT, 2 * ROWS_PER_CORE, 2 * W_IN], mybir.dt.float32, isOutput=True)

    with tile.TileContext(nc) as tc:
        with (
            tc.tile_pool(name="xpool", bufs=MB) as xpool,
            tc.tile_pool(name="wpool", bufs=4) as wpool,
            tc.tile_pool(name="bpool", bufs=1) as bpool,
            tc.tile_pool(name="opool", bufs=4) as opool,
            tc.tile_pool(name="psum", bufs=8, space="PSUM") as pspool,
        ):
            b_sb = bpool.tile([C_OUT, 1], mybir.dt.float32)
            nc.sync.dma_start(out=b_sb[:], in_=bh[:])

            w_sb = []
            for pi in range(4):
                w = wpool.tile([128, 2, 2, 2, C_OUT], _MATMUL_DT, tag="w")
                nc.sync.dma_start(out=w[:], in_=wt[:, pi])
                w_sb.append(w)

            x_sb = []
            for mb in range(MB):
                xt = xpool.tile([128, 2, 6, XW], _MATMUL_DT, tag="x")
                nc.sync.dma_start(out=xt[:], in_=xh[:, :, 4 * mb : 4 * mb + 6, :])
                x_sb.append(xt)

            for mb in range(MB):
                for pi, (ry, rx) in enumerate(PAR):
                    ps = pspool.tile([C_OUT, 4, 128], mybir.dt.float32)
                    i = 0
                    for c in range(2):
                        for dy in range(2):
                            for dx in range(2):
                                r0 = dy + ry
                                c0 = dx + rx
                                rhs = x_sb[mb][:, c, r0 : r0 + 4, c0 : c0 + 128]
                                lhsT = w_sb[pi][:, c, dy, dx, :]
                                nc.tensor.matmul(
                                    ps[:],
                                    lhsT,
                                    rhs,
                                    start=(i == 0),
                                    stop=(i == 7),
                                )
                                i += 1
                    # evict PSUM -> SBUF with bias add, interleaving columns
                    if rx == 0:
                        ot = opool.tile([C_OUT, 4, 2 * W_IN], mybir.dt.float32, tag="o")
                        if ry == 0:
                            ot_even = ot
                        else:
                            ot_odd = ot
                    else:
                        ot = ot_even if ry == 0 else ot_odd
                    dst = ot[:, :, rx : rx + 2 * W_IN - 1 : 2]
                    nc.vector.tensor_scalar_add(dst, ps[:], b_sb[:])
                    if rx == 1:
                        # both column parities of this (mb, ry) stripe done
                        nc.sync.dma_start(
                            out=out[:, 8 * mb + ry : 8 * mb + 8 : 2, :],
                            in_=ot[:],
                        )
    return nc


_NC_CACHE = None


def _get_nc():
    global _NC_CACHE
    if _NC_CACHE is None:
        _NC_CACHE = _build_bass()
    return _NC_CACHE


def _prep_inputs(x, W, b):
    x = np.ascontiguousarray(x, dtype=np.float32)
    W = np.ascontiguousarray(W, dtype=np.float32)
    b = np.ascontiguousarray(b, dtype=np.float32)

    # weights: lhsT[p, pi, c, dy, dx, oc] = W[oc, c*128+p, ky, kx]
    wt = np.zeros((128, 4, 2, 2, 2, C_OUT), dtype=np.float32)
    for pi, (ry, rx) in enumerate(PAR):
        for dy in range(2):
            for dx in range(2):
                ky = 2 * dy + ry
                kx = 2 * dx + rx
                wt[:, pi, :, dy, dx, :] = (
                    W[:, :, ky, kx].T.reshape(2, 128, C_OUT).transpose(1, 0, 2)
                )
    wt = np.ascontiguousarray(wt)

    bh = np.ascontiguousarray(b[:, None])

    in_maps = []
    for r in range(N_CORES):
        lo = ROWS_PER_CORE * r - 1
        xpad = np.zeros((C_IN, 18, XW), dtype=np.float32)
        for riy in range(18):
            iy = lo + riy
            if 0 <= iy < H:
                xpad[:, riy, 1 : 1 + W_IN] = x[:, iy, :]
        xhost = np.ascontiguousarray(
            xpad.reshape(2, 128, 18, XW).transpose(1, 0, 2, 3)
        )
        in_maps.append({"xh": xhost, "wt": wt, "bh": bh})
    return in_maps


def kernel(x, W, b, _trace=False, _result_box=None):
    nc = _get_nc()
    in_maps = _prep_inputs(x, W, b)
    res = run_bass_kernel_spmd(nc, in_maps, core_ids=list(range(N_CORES)), trace=_trace)
    if _result_box is not None:
        _result_box.append(res)
    full = np.concatenate(
        [res.results[r]["out"] for r in range(N_CORES)], axis=1
    )
    return full.astype(np.float32)


# revision 6
# speedup vs baseline: 25.0387x; 25.0387x over previous
"""ConvTranspose2D (stride 2, pad 1, k 4) on 8 Trainium2 NeuronCores.

x: (256, 128, 128) f32, W: (128, 256, 4, 4), b: (128,) -> out: (128, 256, 256)

Decomposition: each output-pixel parity class (row even/odd x col even/odd)
is a dense 2x2-tap stride-1 conv over x. Output rows are sharded across the
8 cores (32 rows each); every core runs the same Bass program on its own
input slice, contraction over C_in=256 runs as 2 chunks of 128 partitions.

  even out rows: out[2m]   = K[0] x[m-1] + K[2] x[m]
  odd  out rows: out[2m+1] = K[1] x[m]   + K[3] x[m+1]
(and identically over columns), so tap (dy,dx) of parity (ry,rx) uses
kernel element ky=2*dy+ry, kx=2*dx+rx at input row m+dy+ry-1,
col n+dx+rx-1 -- handled by a zero-padded per-core input slab.
"""

import json

import numpy as np

import concourse.bass as bass
import concourse.bass2jax as bass2jax
import concourse.bass_utils as bass_utils
import concourse.tile as tile
from concourse import mybir
from concourse.bass_utils import run_bass_kernel_spmd

N_CORES = 8
C_IN = 256
C_OUT = 128
H = 128
W_IN = 128
ROWS_PER_CORE = H // N_CORES          # 16 input rows -> 32 output rows
MB = 4                                # m-blocks per core (4 rows each)
PAR = [(0, 0), (0, 1), (1, 0), (1, 1)]  # (row parity, col parity)
XW = W_IN + 2                         # zero-padded input width

_MATMUL_DT = mybir.dt.float32r


def _split_excess_waits(bir_json: bytes) -> bytes:
    # walrus CoreV3 codegen on this build supports only ONE embedded
    # sync-wait per instruction; Tile emits several. Hoist the excess onto
    # standalone EventSemaphore instructions on the same engine just before
    # the original instruction (same semantics: the sequencer blocks on
    # each in turn).
    bir = json.loads(bir_json)
    changed = False
    for fn in bir.get("functions", []):
        for bb in fn.get("blocks", []):
            out = []
            for inst in bb.get("instructions", []):
                si = inst.get("sync_info")
                if si:
                    ow = si.get("on_wait") or []
                    if len(ow) > 1:
                        for j, w in enumerate(ow[:-1]):
                            out.append({
                                "debug": inst.get("debug", 0),
                                "engine": inst["engine"],
                                "ins": [],
                                "name": f"{inst['name']}-wsplit{j}",
                                "opcode": "EventSemaphore",
                                "outs": [],
                                "sync_info": {"on_update": [], "on_wait": [w]},
                            })
                        si["on_wait"] = ow[-1:]
                        changed = True
                out.append(inst)
            bb["instructions"] = out
    if not changed:
        return bir_json
    return json.dumps(bir).encode()


_orig_compile_bir_kernel = bass_utils.compile_bir_kernel


def _patched_compile_bir_kernel(bir_json, tmpdir, neff_name="file.neff"):
    return _orig_compile_bir_kernel(_split_excess_waits(bir_json), tmpdir, neff_name)


bass_utils.compile_bir_kernel = _patched_compile_bir_kernel
bass2jax.compile_bir_kernel = _patched_compile_bir_kernel


def _build_bass(repeat=1):
    nc = bass.Bass(trn_type="TRN2")
    xh = nc.declare_dram_parameter("xh", [128, 2, 18, XW], _MATMUL_DT, isOutput=False)
    wt = nc.declare_dram_parameter("wt", [128, 4, 2, 2, 2, C_OUT], _MATMUL_DT, isOutput=False)
    bh = nc.declare_dram_parameter("bh", [C_OUT, 1], mybir.dt.float32, isOutput=False)
    out = nc.declare_dram_parameter("out", [C_OUT, 2 * ROWS_PER_CORE, 2 * W_IN], mybir.dt.float32, isOutput=True)

    with tile.TileContext(nc) as tc:
        with (
            tc.tile_pool(name="xpool", bufs=MB + 1) as xpool,
            tc.tile_pool(name="wpool", bufs=4) as wpool,
            tc.tile_pool(name="bpool", bufs=1) as bpool,
            tc.tile_pool(name="opool", bufs=4) as opool,
            tc.tile_pool(name="psum", bufs=8, space="PSUM") as pspool,
        ):
            b_sb = bpool.tile([C_OUT, 1], mybir.dt.float32)
            nc.sync.dma_start(out=b_sb[:], in_=bh[:])

            w_sb = []
            for pi in range(4):
                w = wpool.tile([128, 2, 2, 2, C_OUT], _MATMUL_DT, tag="w")
                nc.sync.dma_start(out=w[:], in_=wt[:, pi])
                w_sb.append(w)

            for _rep in range(repeat):
                x_sb = []
                for mb in range(MB):
                    xt = xpool.tile([128, 2, 6, XW], _MATMUL_DT, tag="x")
                    nc.sync.dma_start(out=xt[:], in_=xh[:, :, 4 * mb : 4 * mb + 6, :])
                    x_sb.append(xt)

                for mb in range(MB):
                    for pi, (ry, rx) in enumerate(PAR):
                        ps = pspool.tile([C_OUT, 4, 128], mybir.dt.float32)
                        i = 0
                        for c in range(2):
                            for dy in range(2):
                                for dx in range(2):
                                    r0 = dy + ry
                                    c0 = dx + rx
                                    rhs = x_sb[mb][:, c, r0 : r0 + 4, c0 : c0 + 128]
                                    lhsT = w_sb[pi][:, c, dy, dx, :]
                                    nc.tensor.matmul(
                                        ps[:],
                                        lhsT,
                                        rhs,
                                        start=(i == 0),
                                        stop=(i == 7),
                                    )
                                    i += 1
                        # evict PSUM -> SBUF with bias add, interleaving columns
                        if rx == 0:
                            ot = opool.tile([C_OUT, 4, 2 * W_IN], mybir.dt.float32, tag="o")
                            if ry == 0:
                                ot_even = ot
                            else:
                                ot_odd = ot
                        else:
                            ot = ot_even if ry == 0 else ot_odd
                        dst = ot[:, :, rx : rx + 2 * W_IN - 1 : 2]
                        nc.vector.tensor_scalar_add(dst, ps[:], b_sb[:])
                        if rx == 1:
                            # both column parities of this (mb, ry) stripe done
                            nc.sync.dma_start(
                                out=out[:, 8 * mb + ry : 8 * mb + 8 : 2, :],
                                in_=ot[:],
                            )
    return nc


_NC_CACHE = None


def _get_nc():
    global _NC_CACHE
    if _NC_CACHE is None:
        _NC_CACHE = _build_bass()
    return _NC_CACHE


def _prep_inputs(x, W, b):
    x = np.ascontiguousarray(x, dtype=np.float32)
    W = np.ascontiguousarray(W, dtype=np.float32)
    b = np.ascontiguousarray(b, dtype=np.float32)

    # weights: lhsT[p, pi, c, dy, dx, oc] = W[oc, c*128+p, ky, kx]
    wt = np.zeros((128, 4, 2, 2, 2, C_OUT), dtype=np.float32)
    for pi, (ry, rx) in enumerate(PAR):
        for dy in range(2):
            for dx in range(2):
                ky = 2 * dy + ry
                kx = 2 * dx + rx
                wt[:, pi, :, dy, dx, :] = (
                    W[:, :, ky, kx].T.reshape(2, 128, C_OUT).transpose(1, 0, 2)
                )
    wt = np.ascontiguousarray(wt)

    bh = np.ascontiguousarray(b[:, None])

    in_maps = []
    for r in range(N_CORES):
        lo = ROWS_PER_CORE * r - 1
        xpad = np.zeros((C_IN, 18, XW), dtype=np.float32)
        for riy in range(18):
            iy = lo + riy
            if 0 <= iy < H:
                xpad[:, riy, 1 : 1 + W_IN] = x[:, iy, :]
        xhost = np.ascontiguousarray(
            xpad.reshape(2, 128, 18, XW).transpose(1, 0, 2, 3)
        )
        in_maps.append({"xh": xhost, "wt": wt, "bh": bh})
    return in_maps


def kernel(x, W, b, _trace=False, _result_box=None):
    nc = _get_nc()
    in_maps = _prep_inputs(x, W, b)
    res = run_bass_kernel_spmd(nc, in_maps, core_ids=list(range(N_CORES)), trace=_trace)
    if _result_box is not None:
        _result_box.append(res)
    full = np.concatenate(
        [res.results[r]["out"] for r in range(N_CORES)], axis=1
    )
    return full.astype(np.float32)
